# revision 32
# baseline (speedup 1.0000x reference)
"""DistanceLoss kernel for Trainium2 (8 NeuronCores, data-parallel over batch).

Computes mean(MARGIN + dist[i, label_i] - min_{c != label_i} dist[i, c]) where
dist is the pairwise L2 distance between row-normalized WO [N, D] and class
embeddings emb [C, D], via the GEMM identity d2 = x2 + e2 - 2 * WOn @ emb.T.

Per core (2048 rows): PSUM = 2*x@E.T - |x|*e2 with x UNNORMALIZED (fp8e4
DoubleRow matmuls; |x|*e2 as a rank-2 fp16 matmul whose lhsT is the per-row
|x| pair and rhs the -e2 hi/lo rows), so min_d2 = 1 - rnorm*max(psum): row
normalization never enters the GEMM datapath (PE transpose mode requires a
permutation rhs, so a diag(rnorm) transpose trick is NOT possible). The min
over classes != label is one custom-DVE TENSOR_MASK_REDUCE per psum tile
using an inverted per-row single-index window (start = col+1 > end = col
selects everything except the label's column) with the two halves chained
through the accum init - exact masked min in a single scan. The label
distance goes through a full-f32 path (indirect-DMA row gather of
emb[label], fused multiply-reduce dot, ScalarE square-accumulate of the
gathered rows), so matmul quantization never touches it. rsqrt/sqrt run
on DVE via bit-trick seed + Newton steps.

Engine balance: DVE runs the masked scans, the dots, |e|^2 for class groups
0-1 (its pre-scan idle window), and small epilogue math; ScalarE runs the
|wo|^2 squares, |e|^2 for groups 2-3, and every PSUM->fp8 evacuation;
GpSimd runs the row gathers (per-row e2 gathers were measured 17% slower
end-to-end than ScalarE squares of the gathered rows: 2048 four-byte SWDGE
descriptors cost more on HW than the cost model knows). Emission is laid
out in explicit per-engine waves:
scans chase their GEMMs group-by-group, the e2 bounce DMAs ride the sync
ring between bulk loads, and the gather offsets come from a second, LATE
label load so 4MB of gather traffic cannot delay the input loads.

Layout tricks: row-block m holds rows {i : i % 16 == m} and class-block c
holds classes {j : j % 16 == c}, which makes every DMA (WO, emb, labels)
contiguous per partition (few large descriptors - descriptor generation, not
bandwidth, dominates DGE cost), at the price of a cheap exact bit-op remap of
the label's matrix column.

Sharding: WO/label split over N across 8 cores, emb replicated; mean on host.
_build(repeat=R) wraps the whole body in a hardware For_i loop (R identical
trips, all-engine barrier between them) - used only by the timing harness.
"""

import sys

if "/opt/trn_rl_repo" not in sys.path:
    sys.path.insert(0, "/opt/trn_rl_repo")

import numpy as np

import concourse.bacc as bacc
import concourse.bass as bass
import concourse.mybir as mybir
import concourse.tile as tile
from concourse.bass_utils import run_bass_kernel_spmd
from concourse.dve_ops import TENSOR_MASK_REDUCE, TENSOR_TENSOR_REDUCE
from concourse.masks import make_identity

MARGIN = 1.0
N_CORES = 8
N_FULL, C, D = 16384, 2048, 512
P = 128
NN = N_FULL // N_CORES          # rows per core (2048)
NT = NN // P                    # row tiles per core (16)
CT = C // P                     # class tiles (16)
KT = D // P                     # contraction tiles (4)
HALF = C // 2                   # psum tile width (1024)

f32 = mybir.dt.float32
f16 = mybir.dt.float16
f8 = mybir.dt.float8e4
i32 = mybir.dt.int32
FP8 = True  # fp8e4 DoubleRow main matmuls (measured end-to-end ~1e-6 rel err)
Alu = mybir.AluOpType
Act = mybir.ActivationFunctionType

NEG_BIG = -3.0e38
QUAKE = 0x5F3759DF


def _rsqrt(nc, pool, x_ap, w, name, iters=3):
    """1/sqrt(x) on DVE: bit-trick seed + Newton. x_ap: [P, w] f32."""
    si = pool.tile([P, w], i32, tag=f"rs_i{name}")
    nc.vector.tensor_scalar(
        out=si[:], in0=x_ap.bitcast(i32), scalar1=1, scalar2=0,
        op0=Alu.logical_shift_right, op1=Alu.bitwise_not,
    )
    nc.vector.tensor_scalar(out=si[:], in0=si[:], scalar1=QUAKE + 1, scalar2=None,
                            op0=Alu.add)
    y = pool.tile([P, w], f32, tag=f"rs_y{name}")
    nc.vector.tensor_copy(out=y[:], in_=si[:].bitcast(f32))
    t = pool.tile([P, w], f32, tag=f"rs_t{name}")
    for _ in range(iters):
        nc.vector.tensor_mul(out=t[:], in0=y[:], in1=y[:])
        nc.vector.tensor_mul(out=t[:], in0=t[:], in1=x_ap)
        nc.vector.tensor_scalar(out=t[:], in0=t[:], scalar1=-0.5, scalar2=1.5,
                                op0=Alu.mult, op1=Alu.add)
        nc.vector.tensor_mul(out=y[:], in0=y[:], in1=t[:])
    return y


def _build(repeat=1):
    nc = bacc.Bacc("TRN2", target_bir_lowering=False, debug=False)

    wo_d = nc.dram_tensor("WO", [NN, D], f32, kind="ExternalInput")
    emb_d = nc.dram_tensor("emb", [C, D], f32, kind="ExternalInput")
    lab_d = nc.dram_tensor("label", [NN, 1], i32, kind="ExternalInput")
    out_d = nc.dram_tensor("out", [P, NT], f32, kind="ExternalOutput")

    from contextlib import nullcontext

    with tile.TileContext(nc) as tc:
        with (
            tc.tile_pool(name="persist", bufs=1) as pp,
            tc.tile_pool(name="elab", bufs=NT) as elp,
            tc.tile_pool(name="sq", bufs=2) as sqp,
            tc.tile_pool(name="tmp", bufs=8) as tmp_p,
            tc.tile_pool(name="mm", bufs=3, space="PSUM") as mmp,
            tc.tile_pool(name="tp", bufs=2, space="PSUM") as tpp,
        ):
            # ---- constants ----
            identf = pp.tile([P, P], f32)
            make_identity(nc, identf[:])
            loop_cm = tc.For_i(0, repeat, 1) if repeat > 1 else nullcontext()
            with loop_cm:
                _emit_body(nc, wo_d, emb_d, lab_d, out_d,
                           pp, elp, sqp, tmp_p, mmp, tpp, identf)

    nc.compile()
    return nc


def _emit_body(nc, wo_d, emb_d, lab_d, out_d,
               pp, elp, sqp, tmp_p, mmp, tpp, identf):
    e2c = pp.tile([P, CT], f32)
    x2 = pp.tile([P, NT], f32)
    rnorm = pp.tile([P, NT], f32)
    e2s_dram = nc.dram_tensor("e2scratch", [1, C], f16)
    e2pair = pp.tile([1, C], f16)
    eT = pp.tile([P, KT, C], f8)
    aT = pp.tile([P, KT, NN], f8)
    e_all = pp.tile([P, CT, D], f32)
    wo_all = pp.tile([P, NT, D], f32)
    emb_v = emb_d.rearrange("(p c) d -> p c d", c=CT)
    wo_v = wo_d.rearrange("(p t) d -> p t d", t=NT)
    lab_v = lab_d[:, 0].rearrange("(p m) -> p m", m=NT)

    negmax = pp.tile([P, NT], f32)
    acc0 = pp.tile([P, NT], f32)
    dots = pp.tile([P, NT], f32)
    elab2 = pp.tile([P, NT], f32)
    elab_tiles = [None] * NT
    # |x| per row, duplicated pairwise and transposed to [2, P] rows per
    # row-tile: lhsT of the e2 rank-2 matmul, so PSUM = 2x.e - |x|*e2 and
    # min_d2 = 1 - rnorm * max(psum). Keeps normalization out of the GEMM
    # datapath entirely (PE transpose mode cannot scale: rhs must be a
    # permutation matrix).
    xn_all = pp.tile([P, NT], f32)
    xnT = [None] * NT
    e2hi = pp.tile([P, CT], f16)
    labi = pp.tile([P, NT], i32)       # early copy: label-window math
    labi_g = pp.tile([P, NT], i32)     # late copy: gather offsets (keeps the
                                       # 4MB of gather traffic off the DMA
                                       # pipe until the bulk loads are done)

    def load(g):
        sl = slice(g * 4, (g + 1) * 4)
        nc.sync.dma_start(out=e_all[:, sl, :], in_=emb_v[:, sl, :])
        nc.scalar.dma_start(out=wo_all[:, sl, :], in_=wo_v[:, sl, :])

    def e2s_bounce(g):
        # -e2 quarter hi/lo to DRAM and back: partition->free transpose.
        # Emitted right after prep_group(g)'s smalls so it rides the sync
        # ring between bulk loads instead of after all of them.
        sl = slice(g * 4, (g + 1) * 4)
        qs = slice(g * 512, (g + 1) * 512)
        nc.sync.dma_start(
            out=e2s_dram[0:1, qs].rearrange("o (ct p) -> o p ct", p=P),
            in_=e2hi[:, sl])
        nc.sync.dma_start(out=e2pair[:, qs], in_=e2s_dram[:, qs])

    def label_smalls():
        # matrix column of class L is (L % CT)*P + L // CT
        # = (label & 15) << 7 | (label >> 4), in exact int bit ops
        lm = tmp_p.tile([P, NT], i32, tag="lm")
        nc.vector.tensor_scalar(out=lm[:], in0=labi[:], scalar1=15, scalar2=7,
                                op0=Alu.bitwise_and, op1=Alu.logical_shift_left)
        ldt = tmp_p.tile([P, NT], i32, tag="ld")
        nc.vector.tensor_scalar(out=ldt[:], in0=labi[:], scalar1=4, scalar2=None,
                                op0=Alu.logical_shift_right)
        nc.vector.tensor_tensor(out=lm[:], in0=lm[:], in1=ldt[:], op=Alu.bitwise_or)
        nc.vector.tensor_copy(out=labj[:], in_=lm[:])
        nc.vector.tensor_scalar_add(out=labf1[:], in0=labj[:], scalar1=1.0)
        nc.vector.tensor_scalar_add(out=labh[:], in0=labj[:], scalar1=float(-HALF))
        nc.vector.tensor_scalar_add(out=labh1[:], in0=labj[:],
                                    scalar1=float(1 - HALF))

    labj = pp.tile([P, NT], f32)       # column index of label class
    labf1 = pp.tile([P, NT], f32)      # col + 1
    labh = pp.tile([P, NT], f32)       # col - HALF
    labh1 = pp.tile([P, NT], f32)      # col - HALF + 1

    pm_tiles = {}

    def mm_mms(h, m):
        pm = mmp.tile([P, HALF], f32, tag="mm", name=f"pm_{h}_{m}")
        pm_tiles[(h, m)] = pm
        for ns in range(2):
            col0 = h * HALF + ns * 512
            for kp in range(0, KT, 2):
                nc.tensor.matmul(
                    out=pm[:, ns * 512 : (ns + 1) * 512],
                    lhsT=aT[:, kp : kp + 2, m * P : (m + 1) * P],
                    rhs=eT[:, kp : kp + 2, col0 : col0 + 512],
                    start=(kp == 0), stop=False,
                    perf_mode=mybir.MatmulPerfMode.DoubleRow,
                )
            nc.tensor.matmul(
                out=pm[:, ns * 512 : (ns + 1) * 512],
                lhsT=xnT[m][:],
                rhs=e2pair[:, col0 : col0 + 512],
                start=False, stop=True,
            )

    def mm_red(h, m):
        st_all = labf1 if h == 0 else labh1
        en_all = labj if h == 0 else labh
        pm = pm_tiles[(h, m)]
        # masked max over c != label (inverted single-index window)
        dmp = tmp_p.tile([P, 1], f32, tag="dmp", name=f"dmp_{h}_{m}")
        nc.vector._custom_dve(
            TENSOR_MASK_REDUCE,
            out=dmp[:].broadcast_to([P, HALF]),
            in0=pm[:],
            in1=en_all[:, m : m + 1],
            s0=st_all[:, m : m + 1],
            s1=NEG_BIG if h == 0 else acc0[:, m : m + 1],
            imm2=1.0,
            accum_out=(acc0 if h == 0 else negmax)[:, m : m + 1],
        )

    # ---- emission: explicit per-engine streams; the tile scheduler keeps
    # per-queue emission order, so each engine's stream is laid out
    # critical-work-first ----
    nc.sync.dma_start(out=labi[:], in_=lab_v)
    load(0)
    load(1)
    nc.sync.dma_start(out=e_all[:, 8:12, :], in_=emb_v[:, 8:12, :])
    nc.sync.dma_start(out=e_all[:, 12:16, :], in_=emb_v[:, 12:16, :])
    nc.scalar.dma_start(out=wo_all[:, 8:12, :], in_=wo_v[:, 8:12, :])
    nc.scalar.dma_start(out=wo_all[:, 12:16, :], in_=wo_v[:, 12:16, :])
    nc.scalar.dma_start(out=labi_g[:], in_=lab_v)
    label_smalls()

    def sq_e(g):
        for t in range(g * 4, (g + 1) * 4):
            s = sqp.tile([P, D], f16, tag="sq", name=f"sq_{t}")
            nc.scalar.activation(out=s[:], in_=e_all[:, t, :], func=Act.Square,
                                 accum_out=e2c[:, t : t + 1])

    def ttr_e(g):
        for t in range(g * 4, (g + 1) * 4):
            de = tmp_p.tile([P, 1], f32, tag="dmp", name=f"dme_{t}")
            nc.vector._custom_dve(
                TENSOR_TENSOR_REDUCE, out=de[:].broadcast_to([P, D]),
                in0=e_all[:, t, :], in1=e_all[:, t, :], s0=0.0, s1=1.0,
                accum_out=e2c[:, t : t + 1],
            )

    def sq_wo(g):
        for t in range(g * 4, (g + 1) * 4):
            sw = sqp.tile([P, D], f16, tag="sq", name=f"sqw_{t}")
            nc.scalar.activation(out=sw[:], in_=wo_all[:, t, :], func=Act.Square,
                                 accum_out=x2[:, t : t + 1])

    def e2smalls(g):
        sl = slice(g * 4, (g + 1) * 4)
        e2n = tmp_p.tile([P, 4], f32, tag="e2n", name=f"e2n_{g}")
        nc.vector.tensor_scalar_mul(out=e2n[:], in0=e2c[:, sl], scalar1=-1.0)
        nc.vector.tensor_copy(out=e2hi[:, sl], in_=e2n[:])
        e2s_bounce(g)

    def rsqrt_d(g):
        sl = slice(g * 4, (g + 1) * 4)
        y = _rsqrt(nc, tmp_p, x2[:, sl], 4, "n", iters=2)
        nc.vector.tensor_scalar_min(out=rnorm[:, sl], in0=y[:], scalar1=1.0e12)
        # |x| = x2 * rsqrt(x2), duplicated into adjacent columns, then one
        # [128, 8] -> [8, 128] transpose + fp16 cast gives this group's four
        # [2, P] lhsT rows for the e2 matmuls
        nc.vector.tensor_mul(out=xn_all[:, sl], in0=x2[:, sl], in1=rnorm[:, sl])
        for m in range(g * 4, (g + 1) * 4):
            tp = tpp.tile([P, KT, P], f32, tag="tp", name=f"tpx_{m}")
            nc.tensor.transpose(out=tp[0:1, 0, :], in_=xn_all[:, m : m + 1],
                                identity=identf[:])
            x16 = pp.tile([1, P], f16, name=f"xnT_{m}")
            xnT[m] = x16
            nc.scalar.copy(out=x16[:], in_=tp[0:1, 0, :])

    def tpe(g):
        for cc in range(g * 4, (g + 1) * 4):
            tp = tpp.tile([P, KT, P], f32, tag="tp", name=f"tpe_{cc}")
            for k in range(KT):
                nc.tensor.transpose(out=tp[:, k, :],
                                    in_=e_all[:, cc, k * P : (k + 1) * P],
                                    identity=identf[:])
            nc.scalar.activation(out=eT[:, :, cc * P : (cc + 1) * P],
                                 in_=tp[:], func=Act.Copy, scale=2.0)

    def tpa(g):
        for m in range(g * 4, (g + 1) * 4):
            tp = tpp.tile([P, KT, P], f32, tag="tp", name=f"tpa_{m}")
            for k in range(KT):
                nc.tensor.transpose(out=tp[:, k, :],
                                    in_=wo_all[:, m, k * P : (k + 1) * P],
                                    identity=identf[:])
            nc.scalar.copy(out=aT[:, :, m * P : (m + 1) * P], in_=tp[:])

    ttr_e(0)
    sq_wo(0)
    ttr_e(1)
    e2smalls(0)
    rsqrt_d(0)
    tpe(0)
    tpe(1)
    tpa(0)
    sq_wo(1)
    e2smalls(1)
    rsqrt_d(1)
    tpa(1)
    for m in range(0, 4):
        mm_mms(0, m)
        mm_red(0, m)
    sq_wo(2)
    rsqrt_d(2)
    tpe(2)
    tpa(2)
    for m in range(4, 8):
        mm_mms(0, m)
        mm_red(0, m)
    sq_wo(3)
    rsqrt_d(3)
    tpa(3)
    for m in range(8, 12):
        mm_mms(0, m)
        mm_red(0, m)
    sq_e(2)
    e2smalls(2)
    tpe(3)
    for m in range(12, 16):
        mm_mms(0, m)
        mm_red(0, m)
    sq_e(3)
    e2smalls(3)
    # label-row gathers: emb[label] rows for the f32 label-distance path.
    # Offsets come from the LATE label copy so gather descriptors cannot
    # post ahead of the bulk loads on the DMA pipe.
    for m in range(NT):
        gt = elp.tile([P, D], f32, tag="elab", name=f"elab_{m}")
        elab_tiles[m] = gt
        nc.gpsimd.indirect_dma_start(
            out=gt[:], out_offset=None, in_=emb_d[:, :],
            in_offset=bass.IndirectOffsetOnAxis(
                ap=labi_g[:, m : m + 1], axis=0),
        )
    def dot(m):
        dmp2 = tmp_p.tile([P, 1], f32, tag="dmp", name=f"dmpd_{m}")
        nc.vector._custom_dve(
            TENSOR_TENSOR_REDUCE, out=dmp2[:].broadcast_to([P, D]),
            in0=wo_all[:, m, :], in1=elab_tiles[m][:], s0=0.0, s1=1.0,
            accum_out=dots[:, m : m + 1],
        )

    # dots slot into the DVE gap between the h0 scans and h1 readiness
    for m in range(NT):
        dot(m)
    for m in range(NT):
        mm_mms(1, m)
        mm_red(1, m)
    # |emb[label]|^2 as ScalarE squares of the gathered rows, at the tail of
    # the Act stream where it overlaps the h1 scans -- replaces 2048 tiny
    # 4-byte SWDGE gather descriptors and the e2 table bounce
    for m in range(NT):
        s = sqp.tile([P, D], f16, tag="sq", name=f"sql_{m}")
        nc.scalar.activation(out=s[:], in_=elab_tiles[m][:], func=Act.Square,
                             accum_out=elab2[:, m : m + 1])

    # ---- epilogue ----
    # label_d2 = 1 + elab2 - 2*rnorm*dot  (x2 of normalized row == 1)
    ld2 = tmp_p.tile([P, NT], f32, tag="ld2")
    nc.vector.tensor_mul(out=ld2[:], in0=rnorm[:], in1=dots[:])
    nc.vector.tensor_scalar(out=ld2[:], in0=ld2[:], scalar1=-2.0, scalar2=1.0,
                            op0=Alu.mult, op1=Alu.add)
    nc.vector.tensor_add(out=ld2[:], in0=ld2[:], in1=elab2[:])
    nc.vector.tensor_scalar_max(out=ld2[:], in0=ld2[:], scalar1=0.0)
    # min_{c!=lab} d2 = 1 - rnorm * negmax (psum carries unnormalized rows)
    md2 = tmp_p.tile([P, NT], f32, tag="md2")
    nc.vector.tensor_mul(out=md2[:], in0=rnorm[:], in1=negmax[:])
    nc.vector.tensor_scalar(out=md2[:], in0=md2[:], scalar1=-1.0, scalar2=1.0,
                            op0=Alu.mult, op1=Alu.add)
    nc.vector.tensor_scalar_max(out=md2[:], in0=md2[:], scalar1=0.0)

    # sqrt(x) = x * rsqrt(x); out = sqrt(ld2) - sqrt(md2)
    rl = _rsqrt(nc, tmp_p, ld2[:], NT, "l", iters=2)
    rm = _rsqrt(nc, tmp_p, md2[:], NT, "m", iters=2)
    nc.vector.tensor_mul(out=rl[:], in0=rl[:], in1=ld2[:])
    nc.vector.tensor_mul(out=rm[:], in0=rm[:], in1=md2[:])
    outv = pp.tile([P, NT], f32)
    nc.vector.tensor_sub(out=outv[:], in0=rl[:], in1=rm[:])
    nc.sync.dma_start(out=out_d[:, :], in_=outv[:])


_NC = None


def kernel(WO, emb_weight, label):
    global _NC
    if _NC is None:
        _NC = _build()

    WO = np.ascontiguousarray(np.asarray(WO, dtype=np.float32))
    emb = np.ascontiguousarray(np.asarray(emb_weight, dtype=np.float32))
    lab = np.asarray(label).astype(np.int32).reshape(N_FULL, 1)

    in_maps = []
    for i in range(N_CORES):
        sl = slice(i * NN, (i + 1) * NN)
        in_maps.append({
            "WO": WO[sl],
            "emb": emb,
            "label": np.ascontiguousarray(lab[sl]),
        })
    res = run_bass_kernel_spmd(_NC, in_maps, core_ids=list(range(N_CORES)))
    vals = np.stack([res.results[i]["out"] for i in range(N_CORES)])
    return np.float32(MARGIN + np.mean(vals.astype(np.float64)))
